# revision 4
# baseline (speedup 1.0000x reference)
"""Trainium2 Bass kernel v4: baseline ap_gather algorithm in raw Bass.

Same gather/product structure as the Tile baseline (clause-per-core, 8
ap_gather s-calls of 8192 idxs, DVE pair products), but hand-scheduled:
the ap_gather ucode library loads first (overlapping x/idx staging), x is
staged with a single DMA from a host-replicated [128, G] feed, and the
Tile framework's per-instruction semaphore overhead is gone.
"""
import os
import sys
import numpy as np

sys.path.insert(0, "/opt/trn_rl_repo")

import concourse.bacc as bacc
import concourse.bass as bass
from concourse import library_config
from concourse import mybir
from concourse.bass_utils import run_bass_kernel_spmd

B, G = 16, 16384
C, S, L = 8, 8, 4
NIDX = 8192          # gathers per Q7 core group per call (4l x 2048g)
GCHUNK = G // 8      # 2048 target atoms per core group

_compiled = None
last_exec_time_ns = None


def _build():
    nc = bacc.Bacc("TRN2", target_bir_lowering=False, debug=False)
    x_d = nc.dram_tensor("x8", [128, G], mybir.dt.float32, kind="ExternalInput")
    idx_d = nc.dram_tensor("idx", [128, S * (NIDX // 16)], mybir.dt.int16,
                           kind="ExternalInput")
    out_d = nc.dram_tensor("out", [128, GCHUNK], mybir.dt.float32,
                           kind="ExternalOutput")

    with (
        nc.Block() as block,
        nc.semaphore() as sX,
        nc.semaphore() as sIdx0,
        nc.semaphore() as sIdx1,
        nc.semaphore() as sG,
        nc.semaphore() as sV,
        nc.semaphore() as sVout,
        nc.semaphore() as sOut,
        nc.sbuf_tensor("x_tile", [128, G], mybir.dt.float32) as x_tile,
        nc.sbuf_tensor("it0", [128, NIDX // 16], mybir.dt.int16) as it0,
        nc.sbuf_tensor("it1", [128, NIDX // 16], mybir.dt.int16) as it1,
        nc.sbuf_tensor("g0", [128, NIDX], mybir.dt.float32) as g0,
        nc.sbuf_tensor("g1", [128, NIDX], mybir.dt.float32) as g1,
        nc.sbuf_tensor("tm1", [128, GCHUNK], mybir.dt.float32) as tm1,
        nc.sbuf_tensor("tm2", [128, GCHUNK], mybir.dt.float32) as tm2,
        nc.sbuf_tensor("tm3", [128, GCHUNK], mybir.dt.float32) as tm3,
        nc.sbuf_tensor("acc", [128, GCHUNK], mybir.dt.float32) as acc,
    ):
        its = [it0, it1]
        gs = [g0, g1]
        sIdxs = [sIdx0, sIdx1]

        @block.sync
        def _(sy: bass.BassEngine):
            sy.dma_start(x_tile[:, :], x_d[:, :]).then_inc(sX, 16)
            for s in range(S):
                if s >= 2:
                    # it[s%2] is read during gather s-2, whose completion
                    # (sG advance) implies the read finished
                    sy.wait_ge(sG, s - 1)
                sy.dma_start(
                    its[s % 2][:, :],
                    idx_d[:, s * (NIDX // 16):(s + 1) * (NIDX // 16)],
                ).then_inc(sIdxs[s % 2], 16)
            sy.wait_ge(sVout, 1)
            sy.dma_start(out_d[:, :], acc[:, :]).then_inc(sOut, 16)
            sy.wait_ge(sOut, 16)

        @block.gpsimd
        def _(gp: bass.BassGpSimd):
            gp.load_library(library_config.ap_gather)
            gp.wait_ge(sX, 16)
            for s in range(S):
                gp.wait_ge(sIdxs[s % 2], 16 * (s // 2 + 1))
                if s >= 2:
                    # g[s%2] free once vector consumed gather s-2
                    gp.wait_ge(sV, s - 1)
                gp.ap_gather(gs[s % 2][:, :], x_tile[:, :], its[s % 2][:, :],
                             channels=128, num_elems=G, d=1,
                             num_idxs=NIDX).then_inc(sG, 1)

        @block.vector
        def _(ve: bass.BassVectorEngine):
            for s in range(S):
                g = gs[s % 2]
                ve.wait_ge(sG, s + 1)

                def A(l):
                    return g[:, l * GCHUNK:(l + 1) * GCHUNK]

                ve.tensor_mul(tm1[:, :], A(0), A(1))
                ve.tensor_mul(tm2[:, :], A(2), A(3)).then_inc(sV, 1)
                ve.drain()
                if s == 0:
                    ve.tensor_mul(acc[:, :], tm1[:, :], tm2[:, :])
                else:
                    ve.tensor_mul(tm3[:, :], tm1[:, :], tm2[:, :])
                    ve.drain()
                    last = ve.tensor_add(acc[:, :], acc[:, :], tm3[:, :])
                    if s == S - 1:
                        last.then_inc(sVout, 1)
                ve.drain()

    nc.compile()
    return nc


def _prep_idx(I: np.ndarray) -> np.ndarray:
    """[C, G, S, L] int64 -> [C, 128, S*512] int16 wrapped ap_gather feed.

    Call s of clause c: core group k gathers, at stream position
    i = l*2048 + w, the atom index I[c, k*2048 + w, s, l]. ap_gather reads
    position i of group k from it[16*k + i%16, i//16].
    """
    T = I.astype(np.int16).reshape(C, 8, GCHUNK, S, L)     # [c,k,w,s,l]
    U = T.transpose(0, 3, 1, 4, 2).reshape(C, S, 8, NIDX)  # [c,s,k,i=l*2048+w]
    W = U.reshape(C, S, 8, NIDX // 16, 16)                 # [c,s,k,col,pp]
    W = W.transpose(0, 2, 4, 1, 3)                         # [c,k,pp,s,col]
    return np.ascontiguousarray(W).reshape(C, 128, S * (NIDX // 16))


def kernel(x: np.ndarray, I: np.ndarray) -> np.ndarray:
    global _compiled, last_exec_time_ns
    if _compiled is None:
        _compiled = _build()
    nc = _compiled

    x = np.ascontiguousarray(np.asarray(x), dtype=np.float32)
    x8 = np.ascontiguousarray(np.tile(x, (8, 1)))          # [128, G]
    idx_feed = _prep_idx(np.asarray(I))

    in_maps = [{"x8": x8, "idx": idx_feed[c]} for c in range(C)]
    kwargs = {}
    if os.environ.get("KERNEL_TRACE") == "1":
        kwargs = {"trace": True, "trace_cores": list(range(C))}
    try:
        res = run_bass_kernel_spmd(nc, in_maps, core_ids=list(range(C)), **kwargs)
    except Exception:
        if not kwargs:
            raise
        # NTFF profiling infra unavailable (e.g. missing antenv.axon_hooks):
        # rerun without tracing rather than failing the kernel call
        res = run_bass_kernel_spmd(nc, in_maps, core_ids=list(range(C)))
    last_exec_time_ns = res.exec_time_ns
    # acc[16k + b, w] = out[b, k*2048 + w]: pure relayout to (B, G)
    out = np.stack(
        [
            res.results[c]["out"].reshape(8, 16, GCHUNK)
            .transpose(1, 0, 2).reshape(B, G)
            for c in range(C)
        ],
        axis=0,
    )
    return np.ascontiguousarray(out, dtype=np.float32)


if __name__ == "__main__":
    rng = np.random.default_rng(0)
    x = rng.random((B, G), dtype=np.float32)
    I = rng.integers(0, G, size=(C, G, S, L)).astype(np.int64)
    out = kernel(x=x, I=I)
    gathered = x[:, I]
    expect = np.moveaxis(np.sum(np.prod(gathered, axis=-1), axis=-1), 0, 1)
    err = np.abs(out - expect).max() / np.abs(expect).max()
    print("exec_time_ns:", last_exec_time_ns)
    print("max rel err:", err)
